# revision 3
# baseline (speedup 1.0000x reference)
"""Trainium2 Bass kernel for nn_LocalMultiheadAttention (sparse_attention).

Strategy (8 cores): shard batch*heads. Core c handles batch b=c//2, heads
4*(c%2) .. 4*(c%2)+4.  Per core, everything is computed in the transposed
[s, t] score layout so that the PV matmul and the row-sum (via an
8.0-augmented column on V) are plain matmuls:

  scoresT[s,t] = sum_d k[s,d] q[t,d]          (PE, fp32, K=64)
  ew = exp(scoresT) (fp16, ACT)  * wpen[s,t]  (DVE fp16, wpen = 1/max(1,|s-t|)
                                               == exp(-max(0, log|s-t|)))
  [attn_aug | 8r] = v_aug^T @ ew              (PE fp16, K=128, v_aug=[v | 8.0])
  probs = ew * bcast(1/(8r))                  (DVE fp16)
  P    += probs  (over 4 heads)               (PE identity-matmul, fp32 PSUM)
  attnT = attn_aug * bcast(1/(8r))            (DVE)  [x8 refolded into Wout]
  outp  = attnT^T @ WoutT                     (PE fp32)

Host: builds transposed/sliced weights, the wpen Toeplitz factor and the
identity; gathers P (summing the two 4-head groups per batch, transposing
[s,t]->[t,s]) and the output projection partials.
"""

import numpy as np
from contextlib import ExitStack

import concourse.bass as bass
import concourse.bacc as bacc
import concourse.tile as tile
from concourse import mybir
from concourse.bass_utils import run_bass_kernel_spmd

F32 = mybir.dt.float32
F16 = mybir.dt.float16

T = 2048          # tgt_len (== src_len)
B = 4             # batch
E = 512           # embed dim
H = 8             # heads total
DH = 64           # head dim
HPC = 4           # heads per core
NCORES = 8
TCH = 512         # t-chunk (psum bank free size, fp32)
NTCH = T // TCH   # 4
SB = 128          # s-block (partitions)
NSB = T // SB     # 16

_cached = {}


def _build_program():
    nc = bacc.Bacc("TRN2", target_bir_lowering=False, debug=False,
                   num_devices=NCORES)

    xT = nc.dram_tensor("xT", [E, T], F32, kind="ExternalInput").ap()
    wT = nc.dram_tensor("wT", [E + 1, 3 * HPC * DH], F32,
                        kind="ExternalInput").ap()        # [513, 768]
    woutT = nc.dram_tensor("woutT", [HPC * DH, E], F32,
                           kind="ExternalInput").ap()     # [256, 512]
    wpen = nc.dram_tensor("wpen", [T, T], F16, kind="ExternalInput").ap()
    ident = nc.dram_tensor("ident", [SB, SB], F16, kind="ExternalInput").ap()

    P = nc.dram_tensor("P", [T, T], F16, kind="ExternalOutput").ap()
    outp = nc.dram_tensor("outp", [T, E], F32, kind="ExternalOutput").ap()

    wpen_r = wpen.rearrange("(a p) (b t) -> p a b t", p=SB, t=TCH)
    P_r = P.rearrange("(a p) (b t) -> p a b t", p=SB, t=TCH)

    with tile.TileContext(nc) as tc, ExitStack() as ctx:
        # PSUM: psc(2) + pattn(4) + pP(2) = 8 banks exactly.
        psc_pool = ctx.enter_context(tc.tile_pool(name="psc", bufs=2, space="PSUM"))
        pattn_pool = ctx.enter_context(tc.tile_pool(name="pattn", bufs=4, space="PSUM"))
        pP_pool = ctx.enter_context(tc.tile_pool(name="pP", bufs=2, space="PSUM"))

        persist = ctx.enter_context(tc.tile_pool(name="persist", bufs=1))

        # ---------------- persistent tiles ----------------
        qk_t = persist.tile([128, 4, T], F32, tag="qk")     # [f 128][ft 4][t]
        v_t = persist.tile([128, NSB, HPC, 66], F16, tag="v")  # v_aug
        wout_t = persist.tile([128, 2, E], F32, tag="wout")
        ident_t = persist.tile([SB, SB], F16, tag="ident")
        attn_t = persist.tile([128, 2, T], F32, tag="attn")  # attnT

        nc.sync.dma_start(out=ident_t, in_=ident)
        nc.sync.dma_start(out=wout_t,
                          in_=woutT.rearrange("(a p) e -> p a e", p=128))

        # ---------------- stage 0: projections (scoped pools) ----------------
        with tc.tile_pool(name="s0", bufs=2) as s0, \
             tc.tile_pool(name="s0w", bufs=1) as s0w:
            w_t = s0w.tile([128, 4, 3 * HPC * DH], F32, tag="w")
            wb_t = s0w.tile([1, 3 * HPC * DH], F32, tag="wb")
            ones_t = s0w.tile([1, T], F32, tag="ones")
            nc.sync.dma_start(out=w_t,
                              in_=wT[0:E, :].rearrange("(a p) f -> p a f", p=128))
            nc.sync.dma_start(out=wb_t, in_=wT[E:E + 1, :])
            nc.vector.memset(ones_t, 1.0)
            nc.gpsimd.memset(v_t[:, :, :, 64:66], 0.0)
            nc.gpsimd.memset(v_t[:, :, :, 64:65], 8.0)

            xT_r = xT.rearrange("(a p) (b t) -> p a b t", p=128, t=TCH)
            for tc4 in range(NTCH):
                x_t = s0.tile([128, 4, TCH], F32, tag="x")
                nc.sync.dma_start(out=x_t, in_=xT_r[:, :, tc4, :])

                # qkT[f, t-chunk] = W_qk^T.T @ xT (+ bias row via K=1 matmul)
                for ft in range(4):
                    ps = psc_pool.tile([128, TCH], F32, tag="psc")
                    for ke in range(4):
                        nc.tensor.matmul(
                            ps,
                            lhsT=w_t[:, ke, ft * 128:(ft + 1) * 128],
                            rhs=x_t[:, ke, :],
                            start=(ke == 0), stop=False)
                    nc.tensor.matmul(
                        ps,
                        lhsT=wb_t[0:1, ft * 128:(ft + 1) * 128],
                        rhs=ones_t[0:1, tc4 * TCH:(tc4 + 1) * TCH],
                        start=False, stop=True)
                    nc.vector.tensor_copy(
                        qk_t[:, ft, tc4 * TCH:(tc4 + 1) * TCH], ps)

                # v natural [s, d] (+ bias) -> v_aug tiles, fp16
                for si in range(4):
                    st = tc4 * 4 + si
                    ps = pP_pool.tile([128, HPC * DH], F32, tag="pP")
                    for ke in range(4):
                        nc.tensor.matmul(
                            ps,
                            lhsT=x_t[:, ke, si * SB:(si + 1) * SB],
                            rhs=w_t[:, ke, 2 * HPC * DH:3 * HPC * DH],
                            start=(ke == 0), stop=False)
                    nc.tensor.matmul(
                        ps,
                        lhsT=ones_t[0:1, st * SB:(st + 1) * SB],
                        rhs=wb_t[0:1, 2 * HPC * DH:3 * HPC * DH],
                        start=False, stop=True)
                    nc.vector.tensor_copy(
                        v_t[:, st, :, 0:64],
                        ps.rearrange("p (h d) -> p h d", h=HPC))

        # ---------------- main-loop pools (opened after stage 0 frees) -----
        wpen_pool = ctx.enter_context(tc.tile_pool(name="wpen", bufs=2))
        ewp_pool = ctx.enter_context(tc.tile_pool(name="ewp", bufs=64))
        small = ctx.enter_context(tc.tile_pool(name="small", bufs=2))
        ib_pool = ctx.enter_context(tc.tile_pool(name="ib", bufs=6))
        pout_pool = ctx.enter_context(tc.tile_pool(name="pout", bufs=3))

        def q_ap(h, tch):
            return qk_t[(h % 2) * 64:(h % 2) * 64 + 64, h // 2,
                        tch * TCH:(tch + 1) * TCH]

        def k_ap(h, st):
            return qk_t[(h % 2) * 64:(h % 2) * 64 + 64, 2 + h // 2,
                        st * SB:(st + 1) * SB]

        for tch in range(NTCH):
            wpen_t = wpen_pool.tile([128, NSB, TCH], F16, tag="wpen")
            nc.sync.dma_start(out=wpen_t, in_=wpen_r[:, :, tch, :])

            ewp_tiles = {}
            invrb16 = {}
            for h in range(HPC):
                pattn = pattn_pool.tile([65, TCH], F32, tag="pattn")
                for st in range(NSB):
                    psc = psc_pool.tile([128, TCH], F32, tag="psc")
                    nc.tensor.matmul(psc, lhsT=k_ap(h, st), rhs=q_ap(h, tch),
                                     start=True, stop=True)
                    ewp = ewp_pool.tile([128, TCH], F16, tag="ewp")
                    nc.scalar.activation(out=ewp, in_=psc,
                                         func=mybir.ActivationFunctionType.Exp)
                    nc.vector.tensor_mul(ewp, ewp, wpen_t[:, st, :])
                    ewp_tiles[(h, st)] = ewp
                    nc.tensor.matmul(pattn, lhsT=v_t[:, st, h, 0:65], rhs=ewp,
                                     start=(st == 0), stop=(st == NSB - 1))
                # r' = 8*r sits in row 64 of pattn; move to SBUF first
                # (reciprocal_approx misreads PSUM at partition offset 64)
                rrow = small.tile([1, TCH], F32, tag="rrow")
                nc.vector.tensor_copy(rrow, pattn[64:65, :])
                invr = small.tile([1, TCH], F32, tag="invr")
                scr = small.tile([1, TCH], F32, tag="scr")
                nc.vector.reciprocal_approx_accurate(
                    out=invr, in_=rrow, scratch=scr)
                invr16 = small.tile([1, TCH], F16, tag="invr16")
                nc.vector.tensor_copy(invr16, invr)
                ib16 = ib_pool.tile([128, TCH], F16, tag="ib")
                nc.gpsimd.partition_broadcast(ib16, invr16[0:1, :])
                invrb16[h] = ib16
                # attnT rows for this head, scaled by 1/(8r) (x8 folded in Wout)
                nc.vector.tensor_mul(
                    attn_t[(h % 2) * 64:(h % 2) * 64 + 64, h // 2,
                           tch * TCH:(tch + 1) * TCH],
                    pattn[0:64, :], ib16[0:64, :])

            # phase B: probs normalize (in place) + head-sum via PE identity
            for st in range(NSB):
                pP = pP_pool.tile([128, TCH], F32, tag="pP")
                for h in range(HPC):
                    ewp = ewp_tiles[(h, st)]
                    nc.vector.tensor_mul(ewp, ewp, invrb16[h])
                    nc.tensor.matmul(pP, lhsT=ident_t, rhs=ewp,
                                     start=(h == 0), stop=(h == HPC - 1))
                po = pout_pool.tile([128, TCH], F16, tag="po")
                nc.vector.tensor_copy(po, pP)
                nc.sync.dma_start(out=P_r[:, st, tch, :], in_=po)

        # ---------------- output projection ----------------
        for mt in range(NSB):
            ps = psc_pool.tile([128, E], F32, tag="psc")
            for kt in range(2):
                nc.tensor.matmul(ps,
                                 lhsT=attn_t[:, kt, mt * SB:(mt + 1) * SB],
                                 rhs=wout_t[:, kt, :],
                                 start=(kt == 0), stop=(kt == 1))
            ot = pout_pool.tile([128, E], F32, tag="po")
            nc.vector.tensor_copy(ot, ps)
            nc.sync.dma_start(out=outp[mt * SB:(mt + 1) * SB, :], in_=ot)

    nc.compile()
    return nc


def _host_prep(query, in_proj_weight, in_proj_bias, out_w):
    """Build the 8 per-core input maps."""
    scaling = DH ** -0.5
    Wq = in_proj_weight[0:E]
    Wk = in_proj_weight[E:2 * E]
    Wv = in_proj_weight[2 * E:3 * E]
    bq, bk, bv = (in_proj_bias[0:E], in_proj_bias[E:2 * E],
                  in_proj_bias[2 * E:3 * E])

    idx = np.arange(T, dtype=np.float32)
    d = np.abs(idx[:, None] - idx[None, :])
    wpen = (1.0 / np.maximum(1.0, d)).astype(np.float16)
    ident = np.eye(SB, dtype=np.float16)

    in_maps = []
    for c in range(NCORES):
        b = c // 2
        rows = slice((c % 2) * HPC * DH, (c % 2) * HPC * DH + HPC * DH)
        wT = np.empty((E + 1, 3 * HPC * DH), dtype=np.float32)
        wT[0:E, 0:256] = (Wq[rows] * scaling).T
        wT[0:E, 256:512] = Wk[rows].T
        wT[0:E, 512:768] = Wv[rows].T
        wT[E, 0:256] = bq[rows] * scaling
        wT[E, 256:512] = bk[rows]
        wT[E, 512:768] = bv[rows]
        in_maps.append({
            "xT": np.ascontiguousarray(query[:, b, :].T),
            "wT": wT,
            "woutT": np.ascontiguousarray(out_w[:, rows].T) * 8.0,
            "wpen": wpen,
            "ident": ident,
        })
    return in_maps


def kernel(query, in_proj_weight, in_proj_bias, out_w, out_b, **run_kwargs):
    query = np.asarray(query, dtype=np.float32)
    in_proj_weight = np.asarray(in_proj_weight, dtype=np.float32)
    in_proj_bias = np.asarray(in_proj_bias, dtype=np.float32)
    out_w = np.asarray(out_w, dtype=np.float32)
    out_b = np.asarray(out_b, dtype=np.float32)

    if "nc" not in _cached:
        _cached["nc"] = _build_program()
    nc = _cached["nc"]

    in_maps = _host_prep(query, in_proj_weight, in_proj_bias, out_w)
    res = run_bass_kernel_spmd(nc, in_maps, core_ids=list(range(NCORES)),
                               **run_kwargs)
    _cached["last_result"] = res

    attn = np.empty((T, B, E), dtype=np.float32)
    avg = np.empty((B, T, T), dtype=np.float32)
    for b in range(B):
        r0, r1 = res.results[2 * b], res.results[2 * b + 1]
        attn[:, b, :] = r0["outp"] + r1["outp"] + out_b
        avg[b] = (r0["P"].astype(np.float32) + r1["P"].astype(np.float32)).T
    return attn, avg


# revision 17
# speedup vs baseline: 17.2505x; 17.2505x over previous
"""Trainium2 Bass kernel for nn_LocalMultiheadAttention (sparse_attention).

Strategy (8 cores): shard batch*heads. Core c handles batch b=c//2, heads
4*(c%2) .. 4*(c%2)+4.  Per core, everything is computed in the transposed
[s, t] score layout so that the PV matmul and the row-sum (via an
8.0-augmented column on V) are plain matmuls:

  scoresT[s,t] = sum_d k[s,d] q[t,d]          (PE, fp32, K=64; consecutive
                                               heads alternate PE row-halves)
  ew = exp(scoresT) (fp16, ACT)  * wpen[s,t]  (DVE/GPSIMD split, fp16;
                                               wpen = 1/max(1,|s-t|)
                                               == exp(-max(0, log|s-t|)))
  [attn_aug | 8r] = v_aug^T @ ew              (PE fp16, K=128, v_aug=[v | 8.0])
  probs = ew * bcast(1/(8r))                  (DVE fp16, in place)
  P    += probs  (over 4 heads)               (PE identity-matmul, fp32 PSUM;
                                               evacuated to fp16 by ACT)
  attnT = attn_aug * bcast(1/(8r))            (DVE)  [x8 refolded into Wout]
  outp  = attnT^T @ WoutT                     (PE fp32)

Host: builds transposed/sliced weights, the wpen Toeplitz factor and the
identity; gathers P (summing the two 4-head groups per batch, transposing
[s,t]->[t,s]) and the output projection partials.
"""

import numpy as np
from contextlib import ExitStack

import concourse.bass as bass
import concourse.bacc as bacc
import concourse.tile as tile
from concourse import mybir
from concourse.bass_utils import run_bass_kernel_spmd

F32 = mybir.dt.float32
F16 = mybir.dt.float16

T = 2048          # tgt_len (== src_len)
B = 4             # batch
E = 512           # embed dim
H = 8             # heads total
DH = 64           # head dim
HPC = 4           # heads per core
NCORES = 8
TCH = 512         # t-chunk (psum bank free size, fp32)
NTCH = T // TCH   # 4
SB = 128          # s-block (partitions)
NSB = T // SB     # 16

_cached = {}


def _build_program(rep=1, use_bias=False, norm_engine="vector"):
    nc = bacc.Bacc("TRN2", target_bir_lowering=False, debug=False,
                   num_devices=NCORES)

    xT = nc.dram_tensor("xT", [E, T], F32, kind="ExternalInput").ap()
    wT = nc.dram_tensor("wT", [E + 1, 3 * HPC * DH], F32,
                        kind="ExternalInput").ap()        # [513, 768]
    woutT = nc.dram_tensor("woutT", [HPC * DH, E], F32,
                           kind="ExternalInput").ap()     # [256, 512]
    wpen = nc.dram_tensor("wpen", [T, T], F16, kind="ExternalInput").ap()
    ident = nc.dram_tensor("ident", [SB, SB], F16, kind="ExternalInput").ap()

    P = nc.dram_tensor("P", [T, T], F16, kind="ExternalOutput").ap()
    outp = nc.dram_tensor("outp", [T, E], F32, kind="ExternalOutput").ap()

    wpen_r = wpen.rearrange("(a p) (b t) -> p a b t", p=SB, t=TCH)
    P_r = P.rearrange("(a p) (b t) -> p a b t", p=SB, t=TCH)

    with tile.TileContext(nc) as tc, ExitStack() as ctx:
        # PSUM: psc(2) + pattn(4) + pP(2) = 8 banks exactly.
        psc_pool = ctx.enter_context(tc.tile_pool(name="psc", bufs=2, space="PSUM"))
        pattn_pool = ctx.enter_context(tc.tile_pool(name="pattn", bufs=4, space="PSUM"))
        pP_pool = ctx.enter_context(tc.tile_pool(name="pP", bufs=2, space="PSUM"))

        persist = ctx.enter_context(tc.tile_pool(name="persist", bufs=1))

        # ---------------- persistent tiles ----------------
        qk_t = persist.tile([128, 4, T], F32, tag="qk")     # [f 128][ft 4][t]
        v_t = persist.tile([128, NSB, HPC, 66], F16, tag="v")  # v_aug
        wout_t = persist.tile([128, 2, E], F32, tag="wout")
        ident_t = persist.tile([SB, SB], F16, tag="ident")
        attn_t = persist.tile([128, 2, T], F32, tag="attn")  # attnT

        nc.sync.dma_start(out=ident_t, in_=ident)
        nc.sync.dma_start(out=wout_t,
                          in_=woutT.rearrange("(a p) e -> p a e", p=128))

        # ---------------- stage 0: projections (scoped pools) ----------------
        with tc.tile_pool(name="s0", bufs=2) as s0, \
             tc.tile_pool(name="s0w", bufs=1) as s0w:
            w_t = s0w.tile([128, 4, 3 * HPC * DH], F32, tag="w")
            wb_t = s0w.tile([1, 3 * HPC * DH], F32, tag="wb")
            ones_t = s0w.tile([1, T], F32, tag="ones")
            nc.sync.dma_start(out=w_t,
                              in_=wT[0:E, :].rearrange("(a p) f -> p a f", p=128))
            nc.sync.dma_start(out=wb_t, in_=wT[E:E + 1, :])
            nc.vector.memset(ones_t, 1.0)
            nc.gpsimd.memset(v_t[:, :, :, 64:66], 0.0)
            nc.gpsimd.memset(v_t[:, :, :, 64:65], 8.0)

            xT_r = xT.rearrange("(a p) (b t) -> p a b t", p=128, t=TCH)
            for tc4 in range(NTCH):
                x_t = s0.tile([128, 4, TCH], F32, tag="x")
                nc.sync.dma_start(out=x_t, in_=xT_r[:, :, tc4, :])

                # qkT[f, t-chunk] = W_qk^T.T @ xT (+ bias row via K=1 matmul)
                for ft in range(4):
                    ps = psc_pool.tile([128, TCH], F32, tag="psc")
                    for ke in range(4):
                        nc.tensor.matmul(
                            ps,
                            lhsT=w_t[:, ke, ft * 128:(ft + 1) * 128],
                            rhs=x_t[:, ke, :],
                            start=(ke == 0), stop=(ke == 3 and not use_bias))
                    if use_bias:
                        nc.tensor.matmul(
                            ps,
                            lhsT=wb_t[0:1, ft * 128:(ft + 1) * 128],
                            rhs=ones_t[0:1, tc4 * TCH:(tc4 + 1) * TCH],
                            start=False, stop=True)
                    nc.vector.tensor_copy(
                        qk_t[:, ft, tc4 * TCH:(tc4 + 1) * TCH], ps)

                # v natural [s, d] (+ bias) -> v_aug tiles, fp16
                for si in range(4):
                    st = tc4 * 4 + si
                    ps = pP_pool.tile([128, HPC * DH], F32, tag="pP")
                    for ke in range(4):
                        nc.tensor.matmul(
                            ps,
                            lhsT=x_t[:, ke, si * SB:(si + 1) * SB],
                            rhs=w_t[:, ke, 2 * HPC * DH:3 * HPC * DH],
                            start=(ke == 0), stop=(ke == 3 and not use_bias))
                    if use_bias:
                        nc.tensor.matmul(
                            ps,
                            lhsT=ones_t[0:1, st * SB:(st + 1) * SB],
                            rhs=wb_t[0:1, 2 * HPC * DH:3 * HPC * DH],
                            start=False, stop=True)
                    nc.vector.tensor_copy(
                        v_t[:, st, :, 0:64],
                        ps.rearrange("p (h d) -> p h d", h=HPC))

        # ---------------- main-loop pools (opened after stage 0 frees) -----
        wpen_pool = ctx.enter_context(tc.tile_pool(name="wpen", bufs=2))
        ewp_pool = ctx.enter_context(tc.tile_pool(name="ewp", bufs=72))
        small = ctx.enter_context(tc.tile_pool(name="small", bufs=2))
        ib_pool = ctx.enter_context(tc.tile_pool(name="ib", bufs=6))
        pout_pool = ctx.enter_context(tc.tile_pool(name="pout", bufs=3))

        def q_ap(h, tch):
            return qk_t[(h % 2) * 64:(h % 2) * 64 + 64, h // 2,
                        tch * TCH:(tch + 1) * TCH]

        def k_ap(h, st):
            return qk_t[(h % 2) * 64:(h % 2) * 64 + 64, 2 + h // 2,
                        st * SB:(st + 1) * SB]

        for tch_rep in range(rep * NTCH):
            tch = tch_rep % NTCH
            wpen_t = wpen_pool.tile([128, NSB, TCH], F16, tag="wpen")
            nc.sync.dma_start(out=wpen_t, in_=wpen_r[:, :, tch, :])

            ewp_tiles = {}
            invrb16 = {}
            pattns = {}
            for h in range(HPC):
                pattn = pattn_pool.tile([65, TCH], F32, tag="pattn")
                pattns[h] = pattn
            # st outer / h inner: consecutive QK matmuls alternate PE
            # row-halves (base partition 0/64) so the PE overlaps them.
            for st in range(NSB):
                for h in range(HPC):
                    psc = psc_pool.tile([128, TCH], F32, tag="psc")
                    nc.tensor.matmul(psc, lhsT=k_ap(h, st), rhs=q_ap(h, tch),
                                     start=True, stop=True)
                    ewp = ewp_pool.tile([128, TCH], F16, tag="ewp")
                    nc.scalar.activation(out=ewp, in_=psc,
                                         func=mybir.ActivationFunctionType.Exp)
                    # split the Toeplitz-penalty multiplies between DVE and
                    # the otherwise-idle GPSIMD engine
                    weng = nc.vector if (st + h) % 2 == 0 else nc.gpsimd
                    weng.tensor_mul(ewp, ewp, wpen_t[:, st, :])
                    ewp_tiles[(h, st)] = ewp
                    nc.tensor.matmul(pattns[h], lhsT=v_t[:, st, h, 0:65],
                                     rhs=ewp,
                                     start=(st == 0), stop=(st == NSB - 1))
            for h in range(HPC):
                pattn = pattns[h]
                # r' = 8*r sits in row 64 of pattn; move to SBUF first
                # (reciprocal_approx misreads PSUM at partition offset 64)
                rrow = small.tile([1, TCH], F32, tag="rrow")
                nc.vector.tensor_copy(rrow, pattn[64:65, :])
                invr = small.tile([1, TCH], F32, tag="invr")
                scr = small.tile([1, TCH], F32, tag="scr")
                nc.vector.reciprocal_approx_accurate(
                    out=invr, in_=rrow, scratch=scr)
                invr16 = small.tile([1, TCH], F16, tag="invr16")
                nc.gpsimd.tensor_copy(invr16, invr)
                ib16 = ib_pool.tile([128, TCH], F16, tag="ib")
                nc.gpsimd.partition_broadcast(ib16, invr16[0:1, :])
                invrb16[h] = ib16
                # attnT rows for this head, scaled by 1/(8r) (x8 folded in Wout)
                nc.vector.tensor_mul(
                    attn_t[(h % 2) * 64:(h % 2) * 64 + 64, h // 2,
                           tch * TCH:(tch + 1) * TCH],
                    pattn[0:64, :], ib16[0:64, :])

            # phase B: probs normalize (in place) + head-sum via PE identity
            norm = nc.gpsimd if norm_engine == "gpsimd" else nc.vector
            for st in range(NSB):
                pP = pP_pool.tile([128, TCH], F32, tag="pP")
                for h in range(HPC):
                    ewp = ewp_tiles[(h, st)]
                    norm.tensor_mul(ewp, ewp, invrb16[h])
                    nc.tensor.matmul(pP, lhsT=ident_t, rhs=ewp,
                                     start=(h == 0), stop=(h == HPC - 1))
                po = pout_pool.tile([128, TCH], F16, tag="po")
                nc.scalar.copy(po, pP)
                nc.sync.dma_start(out=P_r[:, st, tch, :], in_=po)

        # ---------------- output projection ----------------
        for mt in range(NSB):
            ps = psc_pool.tile([128, E], F32, tag="psc")
            for kt in range(2):
                nc.tensor.matmul(ps,
                                 lhsT=attn_t[:, kt, mt * SB:(mt + 1) * SB],
                                 rhs=wout_t[:, kt, :],
                                 start=(kt == 0), stop=(kt == 1))
            ot = pout_pool.tile([128, E], F32, tag="po")
            nc.vector.tensor_copy(ot, ps)
            nc.sync.dma_start(out=outp[mt * SB:(mt + 1) * SB, :], in_=ot)

    nc.compile()
    return nc


def _host_prep(query, in_proj_weight, in_proj_bias, out_w):
    """Build the 8 per-core input maps."""
    scaling = DH ** -0.5
    Wq = in_proj_weight[0:E]
    Wk = in_proj_weight[E:2 * E]
    Wv = in_proj_weight[2 * E:3 * E]
    bq, bk, bv = (in_proj_bias[0:E], in_proj_bias[E:2 * E],
                  in_proj_bias[2 * E:3 * E])

    idx = np.arange(T, dtype=np.float32)
    d = np.abs(idx[:, None] - idx[None, :])
    wpen = (1.0 / np.maximum(1.0, d)).astype(np.float16)
    ident = np.eye(SB, dtype=np.float16)

    in_maps = []
    for c in range(NCORES):
        b = c // 2
        rows = slice((c % 2) * HPC * DH, (c % 2) * HPC * DH + HPC * DH)
        wT = np.empty((E + 1, 3 * HPC * DH), dtype=np.float32)
        wT[0:E, 0:256] = (Wq[rows] * scaling).T
        wT[0:E, 256:512] = Wk[rows].T
        wT[0:E, 512:768] = Wv[rows].T
        wT[E, 0:256] = bq[rows] * scaling
        wT[E, 256:512] = bk[rows]
        wT[E, 512:768] = bv[rows]
        in_maps.append({
            "xT": np.ascontiguousarray(query[:, b, :].T),
            "wT": wT,
            "woutT": np.ascontiguousarray(out_w[:, rows].T) * 8.0,
            "wpen": wpen,
            "ident": ident,
        })
    return in_maps


def kernel(query, in_proj_weight, in_proj_bias, out_w, out_b, **run_kwargs):
    query = np.asarray(query, dtype=np.float32)
    in_proj_weight = np.asarray(in_proj_weight, dtype=np.float32)
    in_proj_bias = np.asarray(in_proj_bias, dtype=np.float32)
    out_w = np.asarray(out_w, dtype=np.float32)
    out_b = np.asarray(out_b, dtype=np.float32)

    use_bias = bool(np.any(in_proj_bias))
    key = ("nc", use_bias)
    if key not in _cached:
        _cached[key] = _build_program(use_bias=use_bias)
    nc = _cached[key]

    in_maps = _host_prep(query, in_proj_weight, in_proj_bias, out_w)
    res = run_bass_kernel_spmd(nc, in_maps, core_ids=list(range(NCORES)),
                               **run_kwargs)
    _cached["last_result"] = res

    attn = np.empty((T, B, E), dtype=np.float32)
    avg = np.empty((B, T, T), dtype=np.float32)
    for b in range(B):
        r0, r1 = res.results[2 * b], res.results[2 * b + 1]
        attn[:, b, :] = r0["outp"] + r1["outp"] + out_b
        avg[b] = (r0["P"].astype(np.float32) + r1["P"].astype(np.float32)).T
    return attn, avg
